# revision 16
# baseline (speedup 1.0000x reference)
"""Trainium2 Bass kernel for nn_NeuralSurface (8-layer MLP SDF with harmonic
embedding + skip concat), data-parallel over 8 NeuronCores.

Layout strategy: activations kept transposed in SBUF ([features, points]),
weights stationary fp16, PE matmuls K/M-chunked to 128. Harmonic sin/cos via
ScalarE Sin LUT (range reduction done host-side). ReLU+bias split between
ScalarE (activation Relu w/ bias) and VectorE (tensor_scalar add+max) reading
PSUM. n-tiles processed in pairs so the PE always has independent matmul work
while ReLUs complete.

v5: rep6 carries frac(x*2^j/2pi) in fp16 (host range reduction; 16x less HBM
traffic than padded-fp32, no DVE rr/ys ops); emb K-padded to 128 via GpSimd
memsets (K<128 matmuls measured +80ns each); packed weights DMA'd in 3 chunks
on the scalar queue; a dummy Sin preloads the trig LUT set before the real
chain needs it; each pair's SDF matmuls are emitted after the NEXT pair's L0
block to deepen the two thinnest relu->matmul shadows; SDF bias/copy runs on
the slack-rich VectorE instead of ScalarE.
"""

import numpy as np

import concourse.bacc as bacc
import concourse.mybir as mybir
import concourse.tile as tile
from concourse.bass_utils import run_bass_kernel_spmd

AF = mybir.ActivationFunctionType
ALU = mybir.AluOpType
F32 = mybir.dt.float32
F16 = mybir.dt.float16

N_CORES = 8
N = 262144
NPC = N // N_CORES  # 32768 points per core
NT = 512  # points per n-tile (PSUM bank / fp32 moving-operand limit)
PAIRS = NPC // (2 * NT)  # 32
H = 256
E = 39
NHARM = 6
TWO_PI = float(2.0 * np.pi)

# packed weight tensor column offsets ([128, WCOLS] fp16; K on partitions)
OFF_W0 = 0
_K_LAYERS = (1, 2, 3, 5, 6, 7)
OFF_WK = {li: 256 + j * 512 for j, li in enumerate(_K_LAYERS)}  # ka, kb halves
OFF_W4E = 256 + 6 * 512  # 3328
OFF_W4A = OFF_W4E + 256
OFF_W4B = OFF_W4A + 256
OFF_SDF = OFF_W4B + 256  # 2 cols: wsdf K-halves a, b
WCOLS = OFF_SDF + 2  # 4098

# ReLU engine split: half 0 -> ACT, half 1 -> DVE (even split; each PSUM
# pair drains through two engines in parallel).
DVE_RELU = {(li, 1): True for li in range(8)}

_CACHED = {}


def _build():
    nc = bacc.Bacc("TRN2")

    rep6 = nc.dram_tensor("rep6", [36, NPC], F16, kind="ExternalInput").ap()
    ptsh = nc.dram_tensor("ptsh", [3, NPC], F16, kind="ExternalInput").ap()
    wpack = nc.dram_tensor("wpack", [128, WCOLS], F16, kind="ExternalInput").ap()
    bmat = nc.dram_tensor("bmat", [128, 16], F32, kind="ExternalInput").ap()
    bsdf1 = nc.dram_tensor("bsdf1", [128, 1], F32, kind="ExternalInput").ap()
    # 2-D output (1-D ExternalOutput tensors fail NEFF load under bass2jax)
    out_o = nc.dram_tensor("out_o", [NPC // NT, NT], F32, kind="ExternalOutput").ap()

    with tile.TileContext(nc) as tc:
        with (
            tc.tile_pool(name="wp", bufs=1) as wp,
            tc.tile_pool(name="ep", bufs=4) as ep,
            tc.tile_pool(name="hp", bufs=4) as hp,
            tc.tile_pool(name="op", bufs=4) as op_,
            tc.tile_pool(name="pp", bufs=6, space="PSUM") as pp,
            tc.tile_pool(name="pf", bufs=1, space="PSUM") as pf,
        ):
            # ---- one-time weight / const loads ----
            # packed weights in 3 chunks on the scalar queue (idle at boot;
            # issues run during the ACT table load), so the sync queue's
            # first issue is pair-0's rep6 (the embedding critical path) and
            # the gpsimd queue only carries the per-pair emb pad memsets.
            zcol = wp.tile([128, 1], F32, name="zcol")
            nc.vector.memset(zcol, 0.0)
            # dummy Sin: forces the trig_and_small LUT set (which also holds
            # Relu/Identity) to load once at boot instead of right before
            # pair-0's sin.
            sindum = wp.tile([36, 1], F16, name="sindum")
            nc.scalar.activation(sindum, zcol[0:36, :], AF.Sin, bias=zcol[0:36, :])
            wps = wp.tile([128, WCOLS], F16, name="wps")
            nc.scalar.dma_start(out=wps[:, 0:512], in_=wpack[:, 0:512])
            nc.scalar.dma_start(out=wps[:, 512:2304], in_=wpack[:, 512:2304])
            nc.scalar.dma_start(out=wps[:, 2304:WCOLS], in_=wpack[:, 2304:WCOLS])
            bms = wp.tile_from(bmat, name="bms")  # [128, 16]
            bsdfs = wp.tile_from(bsdf1, name="bsdfs")  # [1, 1]

            w0s = wps[:, OFF_W0:OFF_W0 + 256]  # [128(39), 256]
            wks = {
                li: (
                    wps[:, OFF_WK[li]:OFF_WK[li] + 256],
                    wps[:, OFF_WK[li] + 256:OFF_WK[li] + 512],
                )
                for li in _K_LAYERS
            }
            w4es = wps[:, OFF_W4E:OFF_W4E + 256]
            w4as = wps[:, OFF_W4A:OFF_W4A + 256]
            w4bs = wps[:, OFF_W4B:OFF_W4B + 256]
            wsdf_a = wps[:, OFF_SDF:OFF_SDF + 1]  # [128, 1]
            wsdf_b = wps[:, OFF_SDF + 1:OFF_SDF + 2]

            # previous pair's state for the deferred SDF emission
            h7_prev = None
            psf_prev = None

            def emit_sdf(h7):
                psfa = pf.tile([1, NT], F32, tag="finA", name="psfa")
                psfb = pf.tile([1, NT], F32, tag="finB", name="psfb")
                nc.tensor.matmul(
                    psfa, wsdf_a, h7[:, bass_ts(0, NT)], start=True, stop=False
                )
                nc.tensor.matmul(
                    psfb, wsdf_a, h7[:, bass_ts(2, NT)], start=True, stop=False
                )
                nc.tensor.matmul(
                    psfa, wsdf_b, h7[:, bass_ts(1, NT)], start=False, stop=True
                )
                nc.tensor.matmul(
                    psfb, wsdf_b, h7[:, bass_ts(3, NT)], start=False, stop=True
                )
                return psfa, psfb

            def emit_sdf_out(pq, psfa, psfb, split=False):
                # bias-add + PSUM->SBUF on VectorE (ScalarE is the busier
                # engine: sins + half the relus); the final pair splits
                # across both engines to shorten the kernel tail.
                oa = op_.tile([1, NT], F32, tag="oa")
                if split:
                    nc.scalar.activation(
                        oa, psfa, AF.Identity, bias=bsdfs[0:1, 0:1]
                    )
                else:
                    nc.vector.tensor_scalar(
                        oa, psfa, bsdfs[0:1, 0:1], None, op0=ALU.add
                    )
                ob = op_.tile([1, NT], F32, tag="ob")
                nc.vector.tensor_scalar(
                    ob, psfb, bsdfs[0:1, 0:1], None, op0=ALU.add
                )
                nc.sync.dma_start(out=out_o[2 * pq:2 * pq + 1, :], in_=oa)
                nc.sync.dma_start(out=out_o[2 * pq + 1:2 * pq + 2, :], in_=ob)

            def emit_embedding(p):
                s = p * 2 * NT
                W = 2 * NT
                # rep6 rows carry y = frac(x*2^j/(2pi) + phase) in [-.5, .5]
                # (host-side fp32 range reduction; 18 sin rows + 18
                # cos-as-phase-shift rows), fp16. emb is two [128, NT] half
                # tiles (rows 39:128 zeroed on GpSimd so L0 runs full-K
                # matmuls; K<128 measured +80ns per matmul) so each L0
                # matmul waits only on its own half; ptsh (rows 36:39) lands
                # independent of the sin (disjoint rows).
                t0 = ep.tile([36, W], F16, tag="t0")
                nc.sync.dma_start(out=t0, in_=rep6[:, s:s + W])
                emb_a = ep.tile([128, NT], F16, tag="embA", name="emb_a")
                emb_b = ep.tile([128, NT], F16, tag="embB", name="emb_b")
                embh = (emb_a, emb_b)
                for hx in range(2):
                    # full-tile memset (GPSIMD partition access must start at
                    # 0); sin/ptsh overwrite rows 0:39 afterwards
                    nc.gpsimd.memset(embh[hx], 0.0)
                    nc.sync.dma_start(
                        out=embh[hx][36:39, :],
                        in_=ptsh[:, s + hx * NT:s + (hx + 1) * NT],
                    )
                    nc.scalar.activation(
                        embh[hx][0:36, :], t0[:, hx * NT:(hx + 1) * NT], AF.Sin,
                        bias=zcol[0:36, :], scale=TWO_PI,
                    )
                return embh

            embh = emit_embedding(0)
            emb_next = None
            for p in range(PAIRS):
                # ---- MLP layers ----
                # h tile layout: [128, 4*NT]: A-half0, A-half1, B-half0, B-half1
                h3 = None
                h_prev = None
                for li in range(8):
                    h = hp.tile([128, 4 * NT], F16, tag="h")
                    # chunks: list of (weight AP, rhs per half_x)
                    if li == 0:
                        chunks = [(w0s, lambda hx: embh[hx])]
                    elif li == 4:
                        chunks = [
                            (w4es, lambda hx: embh[hx]),
                            (w4as, lambda hx, hp3=h3: hp3[:, bass_ts(2 * hx, NT)]),
                            (w4bs, lambda hx, hp3=h3: hp3[:, bass_ts(2 * hx + 1, NT)]),
                        ]
                    else:
                        chunks = [
                            (wks[li][0], lambda hx, hp_=h_prev: hp_[:, bass_ts(2 * hx, NT)]),
                            (wks[li][1], lambda hx, hp_=h_prev: hp_[:, bass_ts(2 * hx + 1, NT)]),
                        ]
                    ps = {(hx, m): pp.tile([128, NT], F32, tag="mm", name="psmm")
                          for hx in range(2) for m in range(2)}
                    last = len(chunks) - 1
                    for hx in range(2):
                        for m in range(2):
                            for ci, (wt, rhs) in enumerate(chunks):
                                nc.tensor.matmul(
                                    ps[(hx, m)], wt[:, bass_ts(m, 128)], rhs(hx),
                                    start=(ci == 0), stop=(ci == last),
                                )
                    # ReLU + bias -> h
                    for half_x in range(2):
                        for m in range(2):
                            dst = h[:, bass_ts(2 * half_x + m, NT)]
                            bias_ap = bms[:, li * 2 + m:li * 2 + m + 1]
                            if DVE_RELU.get((li, m), False):
                                nc.vector.tensor_scalar(
                                    dst, ps[(half_x, m)], bias_ap, 0.0,
                                    op0=ALU.add, op1=ALU.max,
                                )
                            else:
                                nc.scalar.activation(
                                    dst, ps[(half_x, m)], AF.Relu, bias=bias_ap,
                                )
                    if li == 0 and h7_prev is not None:
                        # previous pair's SDF matmuls slot in here: they are
                        # ready to run (h7 relus done) and deepen both the
                        # L0->L1 and L7->SDF relu shadows by 4 matmuls.
                        psf_prev = emit_sdf(h7_prev)
                        h7_prev = None
                    if li == 3:
                        h3 = h
                        if psf_prev is not None:
                            emit_sdf_out(p - 1, *psf_prev)
                            psf_prev = None
                    if li == 5 and p + 1 < PAIRS:
                        # next pair's embedding: emitted here so its sins sit
                        # in the ACT queue ahead of the L6/L7 relus and
                        # complete well before the next pair's L0 matmuls.
                        emb_next = emit_embedding(p + 1)
                    h_prev = h

                h7_prev = h_prev
                embh = emb_next

            psfa, psfb = emit_sdf(h7_prev)
            emit_sdf_out(PAIRS - 1, psfa, psfb, split=True)
    nc.compile()
    return nc


def bass_ts(i, size):
    return slice(i * size, (i + 1) * size)


def _prep_maps(points, ws, bs, wsdf, bsdf):
    pts = np.ascontiguousarray(points, dtype=np.float32).reshape(N, 3)
    freqs = (2.0 ** np.arange(NHARM)).astype(np.float32)
    fcol18 = (np.repeat(freqs[None, :], 3, axis=0).reshape(18, 1) / TWO_PI).astype(
        np.float32
    )

    bmat = np.zeros((128, 16), dtype=np.float32)
    for i in range(8):
        for m in range(2):
            bmat[:, i * 2 + m] = bs[i][m * 128:(m + 1) * 128]

    wpack = np.zeros((128, WCOLS), dtype=np.float16)
    wpack[0:E, OFF_W0:OFF_W0 + 256] = ws[0].astype(np.float16)
    for li in _K_LAYERS:
        wpack[:, OFF_WK[li]:OFF_WK[li] + 256] = ws[li][0:128, :].astype(np.float16)
        wpack[:, OFF_WK[li] + 256:OFF_WK[li] + 512] = ws[li][128:256, :].astype(
            np.float16
        )
    wpack[0:E, OFF_W4E:OFF_W4E + 256] = ws[4][0:E, :].astype(np.float16)
    wpack[:, OFF_W4A:OFF_W4A + 256] = ws[4][E:E + 128, :].astype(np.float16)
    wpack[:, OFF_W4B:OFF_W4B + 256] = ws[4][E + 128:E + 256, :].astype(np.float16)
    wpack[:, OFF_SDF:OFF_SDF + 1] = wsdf[0:128, :].astype(np.float16)
    wpack[:, OFF_SDF + 1:OFF_SDF + 2] = wsdf[128:256, :].astype(np.float16)

    common = {
        "wpack": wpack,
        "bmat": bmat,
        "bsdf1": np.full((128, 1), float(np.ravel(bsdf)[0]), dtype=np.float32),
    }

    in_maps = []
    for c in range(N_CORES):
        sl = pts[c * NPC:(c + 1) * NPC]  # [NPC, 3]
        ptsT = np.ascontiguousarray(sl.T)  # [3, NPC]
        rep3 = np.repeat(ptsT, NHARM, axis=0)  # [18, NPC]
        t18 = rep3 * fcol18  # x * 2^j / (2pi), exact fp32 scaling
        t36 = np.empty((36, NPC), dtype=np.float32)
        t36[0:18], t36[18:36] = t18, t18 + np.float32(0.25)
        # host-side range reduction to [-0.5, 0.5] turns (same fp32 math the
        # kernel's DVE magic-round did); Sin LUT sees scale*y in [-pi, pi]
        rep6 = (t36 - np.round(t36)).astype(np.float16)
        m = dict(common)
        m["rep6"] = rep6
        m["ptsh"] = ptsT.astype(np.float16)
        in_maps.append(m)
    return in_maps


def kernel(
    points, w0, b0, w1, b1, w2, b2, w3, b3, w4, b4, w5, b5, w6, b6, w7, b7,
    wsdf, bsdf,
):
    ws = [np.asarray(w, dtype=np.float32) for w in (w0, w1, w2, w3, w4, w5, w6, w7)]
    bs = [np.asarray(b, dtype=np.float32) for b in (b0, b1, b2, b3, b4, b5, b6, b7)]
    in_maps = _prep_maps(
        np.asarray(points), ws, bs,
        np.asarray(wsdf, dtype=np.float32), np.asarray(bsdf, dtype=np.float32),
    )

    if "nc" not in _CACHED:
        _CACHED["nc"] = _build()
    nc = _CACHED["nc"]

    res = run_bass_kernel_spmd(nc, in_maps, core_ids=list(range(N_CORES)))
    out = np.concatenate(
        [res.results[c]["out_o"] for c in range(N_CORES)], axis=0
    ).reshape(N, 1).astype(np.float32)
    return out


# revision 19
# speedup vs baseline: 1.0034x; 1.0034x over previous
"""Trainium2 Bass kernel for nn_NeuralSurface (8-layer MLP SDF with harmonic
embedding + skip concat), data-parallel over 8 NeuronCores.

Layout strategy: activations kept transposed in SBUF ([features, points]),
weights stationary fp16, PE matmuls K/M-chunked to 128. Harmonic sin/cos via
ScalarE Sin LUT (range reduction done host-side). ReLU+bias split between
ScalarE (activation Relu w/ bias) and VectorE (tensor_scalar add+max) reading
PSUM. n-tiles processed in pairs so the PE always has independent matmul work
while ReLUs complete.

v5: rep6 carries frac(x*2^j/2pi) in fp16 (host range reduction; 16x less HBM
traffic than padded-fp32, no DVE rr/ys ops); emb K-padded to 128 via GpSimd
memsets (K<128 matmuls measured +80ns each); packed weights DMA'd in 3 chunks
on the scalar queue; a dummy Sin preloads the trig LUT set before the real
chain needs it; each pair's SDF matmuls are emitted after the NEXT pair's L0
block to deepen the two thinnest relu->matmul shadows; SDF bias/copy runs on
the slack-rich VectorE instead of ScalarE.
"""

import numpy as np

import concourse.bacc as bacc
import concourse.mybir as mybir
import concourse.tile as tile
from concourse.bass_utils import run_bass_kernel_spmd

AF = mybir.ActivationFunctionType
ALU = mybir.AluOpType
F32 = mybir.dt.float32
F16 = mybir.dt.float16

N_CORES = 8
N = 262144
NPC = N // N_CORES  # 32768 points per core
NT = 512  # points per n-tile (PSUM bank / fp32 moving-operand limit)
PAIRS = NPC // (2 * NT)  # 32
H = 256
E = 39
NHARM = 6
TWO_PI = float(2.0 * np.pi)

# packed weight tensor column offsets ([128, WCOLS] fp16; K on partitions)
OFF_W0 = 0
_K_LAYERS = (1, 2, 3, 5, 6, 7)
OFF_WK = {li: 256 + j * 512 for j, li in enumerate(_K_LAYERS)}  # ka, kb halves
OFF_W4E = 256 + 6 * 512  # 3328
OFF_W4A = OFF_W4E + 256
OFF_W4B = OFF_W4A + 256
OFF_SDF = OFF_W4B + 256  # 2 cols: wsdf K-halves a, b
WCOLS = OFF_SDF + 2  # 4098

# ReLU engine split: half 0 -> ACT, half 1 -> DVE (even split; each PSUM
# pair drains through two engines in parallel).
DVE_RELU = {(li, 1): True for li in range(8)}

_CACHED = {}


def _build():
    nc = bacc.Bacc("TRN2")

    rep6 = nc.dram_tensor("rep6", [36, NPC], F16, kind="ExternalInput").ap()
    ptsh = nc.dram_tensor("ptsh", [3, NPC], F16, kind="ExternalInput").ap()
    wpack = nc.dram_tensor("wpack", [128, WCOLS], F16, kind="ExternalInput").ap()
    bmat = nc.dram_tensor("bmat", [128, 16], F32, kind="ExternalInput").ap()
    bsdf1 = nc.dram_tensor("bsdf1", [128, 1], F32, kind="ExternalInput").ap()
    # 2-D output (1-D ExternalOutput tensors fail NEFF load under bass2jax)
    out_o = nc.dram_tensor("out_o", [NPC // NT, NT], F32, kind="ExternalOutput").ap()

    with tile.TileContext(nc) as tc:
        with (
            tc.tile_pool(name="wp", bufs=1) as wp,
            tc.tile_pool(name="ep", bufs=4) as ep,
            tc.tile_pool(name="hp", bufs=4) as hp,
            tc.tile_pool(name="op", bufs=4) as op_,
            tc.tile_pool(name="pp", bufs=6, space="PSUM") as pp,
            tc.tile_pool(name="pf", bufs=1, space="PSUM") as pf,
        ):
            # ---- one-time weight / const loads ----
            # packed weights in 3 chunks on the scalar queue (idle at boot;
            # issues run during the ACT table load), so the sync queue's
            # first issue is pair-0's rep6 (the embedding critical path) and
            # the gpsimd queue only carries the per-pair emb pad memsets.
            zcol = wp.tile([128, 1], F32, name="zcol")
            nc.vector.memset(zcol, 0.0)
            # dummy Sin: forces the trig_and_small LUT set (which also holds
            # Relu/Identity) to load once at boot instead of right before
            # pair-0's sin.
            sindum = wp.tile([36, 1], F16, name="sindum")
            nc.scalar.activation(sindum, zcol[0:36, :], AF.Sin, bias=zcol[0:36, :])
            # chunk 1 (w0 + all of L1's weights) issues before pair-0's
            # embedding; chunks 2/3 are emitted after it (below) so their
            # transfers don't contend with the pair-0 critical path.
            wps = wp.tile([128, WCOLS], F16, name="wps")
            nc.scalar.dma_start(out=wps[:, 0:1024], in_=wpack[:, 0:1024])
            # bias/bsdf constants ride the gpsimd queue, keeping the sync
            # queue's first issue pair-0's rep6
            bms = wp.tile([128, 16], F32, name="bms")
            nc.gpsimd.dma_start(out=bms, in_=bmat)
            bsdfs = wp.tile([128, 1], F32, name="bsdfs")
            nc.gpsimd.dma_start(out=bsdfs, in_=bsdf1)

            w0s = wps[:, OFF_W0:OFF_W0 + 256]  # [128(39), 256]
            wks = {
                li: (
                    wps[:, OFF_WK[li]:OFF_WK[li] + 256],
                    wps[:, OFF_WK[li] + 256:OFF_WK[li] + 512],
                )
                for li in _K_LAYERS
            }
            w4es = wps[:, OFF_W4E:OFF_W4E + 256]
            w4as = wps[:, OFF_W4A:OFF_W4A + 256]
            w4bs = wps[:, OFF_W4B:OFF_W4B + 256]
            wsdf_a = wps[:, OFF_SDF:OFF_SDF + 1]  # [128, 1]
            wsdf_b = wps[:, OFF_SDF + 1:OFF_SDF + 2]

            # previous pair's state for the deferred SDF emission
            h7_prev = None
            psf_prev = None

            def emit_sdf(h7):
                psfa = pf.tile([1, NT], F32, tag="finA", name="psfa")
                psfb = pf.tile([1, NT], F32, tag="finB", name="psfb")
                nc.tensor.matmul(
                    psfa, wsdf_a, h7[:, bass_ts(0, NT)], start=True, stop=False
                )
                nc.tensor.matmul(
                    psfb, wsdf_a, h7[:, bass_ts(2, NT)], start=True, stop=False
                )
                nc.tensor.matmul(
                    psfa, wsdf_b, h7[:, bass_ts(1, NT)], start=False, stop=True
                )
                nc.tensor.matmul(
                    psfb, wsdf_b, h7[:, bass_ts(3, NT)], start=False, stop=True
                )
                return psfa, psfb

            def emit_sdf_out(pq, psfa, psfb, split=False):
                # bias-add + PSUM->SBUF on VectorE (ScalarE is the busier
                # engine: sins + half the relus); the final pair splits
                # across both engines to shorten the kernel tail.
                oa = op_.tile([1, NT], F32, tag="oa")
                if split:
                    nc.scalar.activation(
                        oa, psfa, AF.Identity, bias=bsdfs[0:1, 0:1]
                    )
                else:
                    nc.vector.tensor_scalar(
                        oa, psfa, bsdfs[0:1, 0:1], None, op0=ALU.add
                    )
                ob = op_.tile([1, NT], F32, tag="ob")
                nc.vector.tensor_scalar(
                    ob, psfb, bsdfs[0:1, 0:1], None, op0=ALU.add
                )
                nc.sync.dma_start(out=out_o[2 * pq:2 * pq + 1, :], in_=oa)
                nc.sync.dma_start(out=out_o[2 * pq + 1:2 * pq + 2, :], in_=ob)

            def emit_embedding(p):
                s = p * 2 * NT
                W = 2 * NT
                # rep6 rows carry y = frac(x*2^j/(2pi) + phase) in [-.5, .5]
                # (host-side fp32 range reduction; 18 sin rows + 18
                # cos-as-phase-shift rows), fp16. emb is two [128, NT] half
                # tiles (rows 39:128 zeroed on GpSimd so L0 runs full-K
                # matmuls; K<128 measured +80ns per matmul) so each L0
                # matmul waits only on its own half; ptsh (rows 36:39) lands
                # independent of the sin (disjoint rows).
                t0 = ep.tile([36, W], F16, tag="t0")
                if p == 0:
                    # split halves: sin A starts as soon as its half lands
                    nc.sync.dma_start(out=t0[:, 0:NT], in_=rep6[:, s:s + NT])
                    nc.sync.dma_start(out=t0[:, NT:W], in_=rep6[:, s + NT:s + W])
                else:
                    nc.sync.dma_start(out=t0, in_=rep6[:, s:s + W])
                emb_a = ep.tile([128, NT], F16, tag="embA", name="emb_a")
                emb_b = ep.tile([128, NT], F16, tag="embB", name="emb_b")
                embh = (emb_a, emb_b)
                for hx in range(2):
                    # full-tile memset (GPSIMD partition access must start at
                    # 0); sin/ptsh overwrite rows 0:39 afterwards
                    nc.gpsimd.memset(embh[hx], 0.0)
                    nc.sync.dma_start(
                        out=embh[hx][36:39, :],
                        in_=ptsh[:, s + hx * NT:s + (hx + 1) * NT],
                    )
                    nc.scalar.activation(
                        embh[hx][0:36, :], t0[:, hx * NT:(hx + 1) * NT], AF.Sin,
                        bias=zcol[0:36, :], scale=TWO_PI,
                    )
                return embh

            embh = emit_embedding(0)
            # remaining weight chunks: issued after pair-0's sins on the
            # scalar queue; wk2b..wk6a land before L2 of pair 0 needs them
            nc.scalar.dma_start(out=wps[:, 1024:2560], in_=wpack[:, 1024:2560])
            nc.scalar.dma_start(out=wps[:, 2560:WCOLS], in_=wpack[:, 2560:WCOLS])
            emb_next = None
            for p in range(PAIRS):
                # ---- MLP layers ----
                # h tile layout: [128, 4*NT]: A-half0, A-half1, B-half0, B-half1
                h3 = None
                h_prev = None
                for li in range(8):
                    h = hp.tile([128, 4 * NT], F16, tag="h")
                    # chunks: list of (weight AP, rhs per half_x)
                    if li == 0:
                        chunks = [(w0s, lambda hx: embh[hx])]
                    elif li == 4:
                        chunks = [
                            (w4es, lambda hx: embh[hx]),
                            (w4as, lambda hx, hp3=h3: hp3[:, bass_ts(2 * hx, NT)]),
                            (w4bs, lambda hx, hp3=h3: hp3[:, bass_ts(2 * hx + 1, NT)]),
                        ]
                    else:
                        chunks = [
                            (wks[li][0], lambda hx, hp_=h_prev: hp_[:, bass_ts(2 * hx, NT)]),
                            (wks[li][1], lambda hx, hp_=h_prev: hp_[:, bass_ts(2 * hx + 1, NT)]),
                        ]
                    ps = {(hx, m): pp.tile([128, NT], F32, tag="mm", name="psmm")
                          for hx in range(2) for m in range(2)}
                    last = len(chunks) - 1
                    for hx in range(2):
                        for m in range(2):
                            for ci, (wt, rhs) in enumerate(chunks):
                                nc.tensor.matmul(
                                    ps[(hx, m)], wt[:, bass_ts(m, 128)], rhs(hx),
                                    start=(ci == 0), stop=(ci == last),
                                )
                    # ReLU + bias -> h
                    for half_x in range(2):
                        for m in range(2):
                            dst = h[:, bass_ts(2 * half_x + m, NT)]
                            bias_ap = bms[:, li * 2 + m:li * 2 + m + 1]
                            if DVE_RELU.get((li, m), False):
                                nc.vector.tensor_scalar(
                                    dst, ps[(half_x, m)], bias_ap, 0.0,
                                    op0=ALU.add, op1=ALU.max,
                                )
                            else:
                                nc.scalar.activation(
                                    dst, ps[(half_x, m)], AF.Relu, bias=bias_ap,
                                )
                    if li == 0 and h7_prev is not None:
                        # previous pair's SDF matmuls slot in here: they are
                        # ready to run (h7 relus done) and deepen both the
                        # L0->L1 and L7->SDF relu shadows by 4 matmuls.
                        psf_prev = emit_sdf(h7_prev)
                        h7_prev = None
                    if li == 3:
                        h3 = h
                        if psf_prev is not None:
                            emit_sdf_out(p - 1, *psf_prev)
                            psf_prev = None
                    if li == 5 and p + 1 < PAIRS:
                        # next pair's embedding: emitted here so its sins sit
                        # in the ACT queue ahead of the L6/L7 relus and
                        # complete well before the next pair's L0 matmuls.
                        emb_next = emit_embedding(p + 1)
                    h_prev = h

                h7_prev = h_prev
                embh = emb_next

            psfa, psfb = emit_sdf(h7_prev)
            emit_sdf_out(PAIRS - 1, psfa, psfb, split=True)
    nc.compile()
    return nc


def bass_ts(i, size):
    return slice(i * size, (i + 1) * size)


def _prep_maps(points, ws, bs, wsdf, bsdf):
    pts = np.ascontiguousarray(points, dtype=np.float32).reshape(N, 3)
    freqs = (2.0 ** np.arange(NHARM)).astype(np.float32)
    fcol18 = (np.repeat(freqs[None, :], 3, axis=0).reshape(18, 1) / TWO_PI).astype(
        np.float32
    )

    bmat = np.zeros((128, 16), dtype=np.float32)
    for i in range(8):
        for m in range(2):
            bmat[:, i * 2 + m] = bs[i][m * 128:(m + 1) * 128]

    wpack = np.zeros((128, WCOLS), dtype=np.float16)
    wpack[0:E, OFF_W0:OFF_W0 + 256] = ws[0].astype(np.float16)
    for li in _K_LAYERS:
        wpack[:, OFF_WK[li]:OFF_WK[li] + 256] = ws[li][0:128, :].astype(np.float16)
        wpack[:, OFF_WK[li] + 256:OFF_WK[li] + 512] = ws[li][128:256, :].astype(
            np.float16
        )
    wpack[0:E, OFF_W4E:OFF_W4E + 256] = ws[4][0:E, :].astype(np.float16)
    wpack[:, OFF_W4A:OFF_W4A + 256] = ws[4][E:E + 128, :].astype(np.float16)
    wpack[:, OFF_W4B:OFF_W4B + 256] = ws[4][E + 128:E + 256, :].astype(np.float16)
    wpack[:, OFF_SDF:OFF_SDF + 1] = wsdf[0:128, :].astype(np.float16)
    wpack[:, OFF_SDF + 1:OFF_SDF + 2] = wsdf[128:256, :].astype(np.float16)

    common = {
        "wpack": wpack,
        "bmat": bmat,
        "bsdf1": np.full((128, 1), float(np.ravel(bsdf)[0]), dtype=np.float32),
    }

    in_maps = []
    for c in range(N_CORES):
        sl = pts[c * NPC:(c + 1) * NPC]  # [NPC, 3]
        ptsT = np.ascontiguousarray(sl.T)  # [3, NPC]
        rep3 = np.repeat(ptsT, NHARM, axis=0)  # [18, NPC]
        t18 = rep3 * fcol18  # x * 2^j / (2pi), exact fp32 scaling
        t36 = np.empty((36, NPC), dtype=np.float32)
        t36[0:18], t36[18:36] = t18, t18 + np.float32(0.25)
        # host-side range reduction to [-0.5, 0.5] turns (same fp32 math the
        # kernel's DVE magic-round did); Sin LUT sees scale*y in [-pi, pi]
        rep6 = (t36 - np.round(t36)).astype(np.float16)
        m = dict(common)
        m["rep6"] = rep6
        m["ptsh"] = ptsT.astype(np.float16)
        in_maps.append(m)
    return in_maps


def kernel(
    points, w0, b0, w1, b1, w2, b2, w3, b3, w4, b4, w5, b5, w6, b6, w7, b7,
    wsdf, bsdf,
):
    ws = [np.asarray(w, dtype=np.float32) for w in (w0, w1, w2, w3, w4, w5, w6, w7)]
    bs = [np.asarray(b, dtype=np.float32) for b in (b0, b1, b2, b3, b4, b5, b6, b7)]
    in_maps = _prep_maps(
        np.asarray(points), ws, bs,
        np.asarray(wsdf, dtype=np.float32), np.asarray(bsdf, dtype=np.float32),
    )

    if "nc" not in _CACHED:
        _CACHED["nc"] = _build()
    nc = _CACHED["nc"]

    res = run_bass_kernel_spmd(nc, in_maps, core_ids=list(range(N_CORES)))
    out = np.concatenate(
        [res.results[c]["out_o"] for c in range(N_CORES)], axis=0
    ).reshape(N, 1).astype(np.float32)
    return out


# revision 22
# speedup vs baseline: 1.0078x; 1.0044x over previous
"""Trainium2 Bass kernel for nn_NeuralSurface (8-layer MLP SDF with harmonic
embedding + skip concat), data-parallel over 8 NeuronCores.

Layout strategy: activations kept transposed in SBUF ([features, points]),
weights stationary fp16, PE matmuls K/M-chunked to 128. Harmonic sin/cos via
ScalarE Sin LUT (range reduction done host-side). ReLU+bias split between
ScalarE (activation Relu w/ bias) and VectorE (tensor_scalar add+max) reading
PSUM. n-tiles processed in pairs so the PE always has independent matmul work
while ReLUs complete.

v5: rep6 carries frac(x*2^j/2pi) in fp16 (host range reduction; 16x less HBM
traffic than padded-fp32, no DVE rr/ys ops); emb K-padded to 128 via GpSimd
memsets (K<128 matmuls measured +80ns each); packed weights DMA'd in 3 chunks
on the scalar queue; a dummy Sin preloads the trig LUT set before the real
chain needs it; each pair's SDF matmuls are emitted after the NEXT pair's L0
block to deepen the two thinnest relu->matmul shadows; SDF bias/copy runs on
the slack-rich VectorE instead of ScalarE.
"""

import numpy as np

import concourse.bacc as bacc
import concourse.mybir as mybir
import concourse.tile as tile
from concourse.bass_utils import run_bass_kernel_spmd

AF = mybir.ActivationFunctionType
ALU = mybir.AluOpType
F32 = mybir.dt.float32
F16 = mybir.dt.float16

N_CORES = 8
N = 262144
NPC = N // N_CORES  # 32768 points per core
NT = 512  # points per n-tile (PSUM bank / fp32 moving-operand limit)
PAIRS = NPC // (2 * NT)  # 32
H = 256
E = 39
NHARM = 6
TWO_PI = float(2.0 * np.pi)

# packed weight tensor column offsets ([128, WCOLS] fp16; K on partitions)
OFF_W0 = 0
_K_LAYERS = (1, 2, 3, 5, 6, 7)
OFF_WK = {li: 256 + j * 512 for j, li in enumerate(_K_LAYERS)}  # ka, kb halves
OFF_W4E = 256 + 6 * 512  # 3328
OFF_W4A = OFF_W4E + 256
OFF_W4B = OFF_W4A + 256
OFF_SDF = OFF_W4B + 256  # 2 cols: wsdf K-halves a, b
WCOLS = OFF_SDF + 2  # 4098

# ReLU engine split: half 0 -> ACT, half 1 -> DVE (even split; each PSUM
# pair drains through two engines in parallel).
DVE_RELU = {(li, 1): True for li in range(8)}

_CACHED = {}


def _build():
    nc = bacc.Bacc("TRN2")

    rep6 = nc.dram_tensor("rep6", [36, NPC], F16, kind="ExternalInput").ap()
    ptsh = nc.dram_tensor("ptsh", [3, NPC], F16, kind="ExternalInput").ap()
    wpack = nc.dram_tensor("wpack", [128, WCOLS], F16, kind="ExternalInput").ap()
    bmat = nc.dram_tensor("bmat", [128, 16], F32, kind="ExternalInput").ap()
    bsdf1 = nc.dram_tensor("bsdf1", [128, 1], F32, kind="ExternalInput").ap()
    # 2-D output (1-D ExternalOutput tensors fail NEFF load under bass2jax)
    out_o = nc.dram_tensor("out_o", [NPC // NT, NT], F32, kind="ExternalOutput").ap()

    with tile.TileContext(nc) as tc:
        with (
            tc.tile_pool(name="wp", bufs=1) as wp,
            tc.tile_pool(name="ep", bufs=4) as ep,
            tc.tile_pool(name="hp", bufs=4) as hp,
            tc.tile_pool(name="op", bufs=4) as op_,
            tc.tile_pool(name="pp", bufs=8, space="PSUM") as pp,
        ):
            # ---- one-time weight / const loads ----
            # packed weights in 3 chunks on the scalar queue (idle at boot;
            # issues run during the ACT table load), so the sync queue's
            # first issue is pair-0's rep6 (the embedding critical path) and
            # the gpsimd queue only carries the per-pair emb pad memsets.
            zcol = wp.tile([128, 1], F32, name="zcol")
            nc.vector.memset(zcol, 0.0)
            # dummy Sin: forces the trig_and_small LUT set (which also holds
            # Relu/Identity) to load once at boot instead of right before
            # pair-0's sin.
            sindum = wp.tile([36, 1], F16, name="sindum")
            nc.scalar.activation(sindum, zcol[0:36, :], AF.Sin, bias=zcol[0:36, :])
            # chunk 1 (w0 + all of L1's weights) issues before pair-0's
            # embedding; chunks 2/3 are emitted after it (below) so their
            # transfers don't contend with the pair-0 critical path.
            wps = wp.tile([128, WCOLS], F16, name="wps")
            nc.scalar.dma_start(out=wps[:, 0:1024], in_=wpack[:, 0:1024])
            # bias/bsdf constants ride the gpsimd queue, keeping the sync
            # queue's first issue pair-0's rep6
            bms = wp.tile([128, 16], F32, name="bms")
            nc.gpsimd.dma_start(out=bms, in_=bmat)
            bsdfs = wp.tile([128, 1], F32, name="bsdfs")
            nc.gpsimd.dma_start(out=bsdfs, in_=bsdf1)

            w0s = wps[:, OFF_W0:OFF_W0 + 256]  # [128(39), 256]
            wks = {
                li: (
                    wps[:, OFF_WK[li]:OFF_WK[li] + 256],
                    wps[:, OFF_WK[li] + 256:OFF_WK[li] + 512],
                )
                for li in _K_LAYERS
            }
            w4es = wps[:, OFF_W4E:OFF_W4E + 256]
            w4as = wps[:, OFF_W4A:OFF_W4A + 256]
            w4bs = wps[:, OFF_W4B:OFF_W4B + 256]
            wsdf_a = wps[:, OFF_SDF:OFF_SDF + 1]  # [128, 1]
            wsdf_b = wps[:, OFF_SDF + 1:OFF_SDF + 2]

            # previous pair's state for the deferred SDF emission
            h7_prev = None
            psf_prev = None

            def emit_sdf(h7):
                # SDF PSUM tiles share the main ring: 8 banks of reuse
                # distance instead of a 6+2 split (bank-WAR stalls measured
                # at 5 fixed positions per pair with the shallower ring)
                psfa = pp.tile([1, NT], F32, tag="mm", name="psfa")
                psfb = pp.tile([1, NT], F32, tag="mm", name="psfb")
                nc.tensor.matmul(
                    psfa, wsdf_a, h7[:, bass_ts(0, NT)], start=True, stop=False
                )
                nc.tensor.matmul(
                    psfb, wsdf_a, h7[:, bass_ts(2, NT)], start=True, stop=False
                )
                nc.tensor.matmul(
                    psfa, wsdf_b, h7[:, bass_ts(1, NT)], start=False, stop=True
                )
                nc.tensor.matmul(
                    psfb, wsdf_b, h7[:, bass_ts(3, NT)], start=False, stop=True
                )
                return psfa, psfb

            def emit_sdf_out(pq, psfa, psfb, split=False):
                # bias-add + PSUM->SBUF on VectorE (ScalarE is the busier
                # engine: sins + half the relus); the final pair splits
                # across both engines to shorten the kernel tail.
                oa = op_.tile([1, NT], F32, tag="oa")
                if split:
                    nc.scalar.activation(
                        oa, psfa, AF.Identity, bias=bsdfs[0:1, 0:1]
                    )
                else:
                    nc.vector.tensor_scalar(
                        oa, psfa, bsdfs[0:1, 0:1], None, op0=ALU.add
                    )
                ob = op_.tile([1, NT], F32, tag="ob")
                nc.vector.tensor_scalar(
                    ob, psfb, bsdfs[0:1, 0:1], None, op0=ALU.add
                )
                nc.sync.dma_start(out=out_o[2 * pq:2 * pq + 1, :], in_=oa)
                nc.sync.dma_start(out=out_o[2 * pq + 1:2 * pq + 2, :], in_=ob)

            def emit_embedding(p):
                s = p * 2 * NT
                W = 2 * NT
                # rep6 rows carry y = frac(x*2^j/(2pi) + phase) in [-.5, .5]
                # (host-side fp32 range reduction; 18 sin rows + 18
                # cos-as-phase-shift rows), fp16. emb is two [128, NT] half
                # tiles (rows 39:128 zeroed on GpSimd so L0 runs full-K
                # matmuls; K<128 measured +80ns per matmul) so each L0
                # matmul waits only on its own half; ptsh (rows 36:39) lands
                # independent of the sin (disjoint rows).
                t0 = ep.tile([36, W], F16, tag="t0")
                if p == 0:
                    # split halves: sin A starts as soon as its half lands
                    nc.sync.dma_start(out=t0[:, 0:NT], in_=rep6[:, s:s + NT])
                    nc.sync.dma_start(out=t0[:, NT:W], in_=rep6[:, s + NT:s + W])
                else:
                    nc.sync.dma_start(out=t0, in_=rep6[:, s:s + W])
                emb_a = ep.tile([128, NT], F16, tag="embA", name="emb_a")
                emb_b = ep.tile([128, NT], F16, tag="embB", name="emb_b")
                embh = (emb_a, emb_b)
                for hx in range(2):
                    # full-tile memset (GPSIMD partition access must start at
                    # 0); sin/ptsh overwrite rows 0:39 afterwards
                    nc.gpsimd.memset(embh[hx], 0.0)
                    nc.sync.dma_start(
                        out=embh[hx][36:39, :],
                        in_=ptsh[:, s + hx * NT:s + (hx + 1) * NT],
                    )
                    nc.scalar.activation(
                        embh[hx][0:36, :], t0[:, hx * NT:(hx + 1) * NT], AF.Sin,
                        bias=zcol[0:36, :], scale=TWO_PI,
                    )
                return embh

            embh = emit_embedding(0)
            # remaining weight chunks: issued after pair-0's sins on the
            # scalar queue; wk2b..wk6a land before L2 of pair 0 needs them
            nc.scalar.dma_start(out=wps[:, 1024:2560], in_=wpack[:, 1024:2560])
            nc.scalar.dma_start(out=wps[:, 2560:WCOLS], in_=wpack[:, 2560:WCOLS])
            emb_next = None
            for p in range(PAIRS):
                # ---- MLP layers ----
                # h tile layout: [128, 4*NT]: A-half0, A-half1, B-half0, B-half1
                h3 = None
                h_prev = None
                for li in range(8):
                    h = hp.tile([128, 4 * NT], F16, tag="h")
                    # chunks: list of (weight AP, rhs per half_x)
                    if li == 0:
                        chunks = [(w0s, lambda hx: embh[hx])]
                    elif li == 4:
                        chunks = [
                            (w4es, lambda hx: embh[hx]),
                            (w4as, lambda hx, hp3=h3: hp3[:, bass_ts(2 * hx, NT)]),
                            (w4bs, lambda hx, hp3=h3: hp3[:, bass_ts(2 * hx + 1, NT)]),
                        ]
                    else:
                        chunks = [
                            (wks[li][0], lambda hx, hp_=h_prev: hp_[:, bass_ts(2 * hx, NT)]),
                            (wks[li][1], lambda hx, hp_=h_prev: hp_[:, bass_ts(2 * hx + 1, NT)]),
                        ]
                    ps = {(hx, m): pp.tile([128, NT], F32, tag="mm", name="psmm")
                          for hx in range(2) for m in range(2)}
                    last = len(chunks) - 1
                    for hx in range(2):
                        for m in range(2):
                            for ci, (wt, rhs) in enumerate(chunks):
                                nc.tensor.matmul(
                                    ps[(hx, m)], wt[:, bass_ts(m, 128)], rhs(hx),
                                    start=(ci == 0), stop=(ci == last),
                                )
                    # ReLU + bias -> h
                    for half_x in range(2):
                        for m in range(2):
                            dst = h[:, bass_ts(2 * half_x + m, NT)]
                            bias_ap = bms[:, li * 2 + m:li * 2 + m + 1]
                            if DVE_RELU.get((li, m), False):
                                nc.vector.tensor_scalar(
                                    dst, ps[(half_x, m)], bias_ap, 0.0,
                                    op0=ALU.add, op1=ALU.max,
                                )
                            else:
                                nc.scalar.activation(
                                    dst, ps[(half_x, m)], AF.Relu, bias=bias_ap,
                                )
                    if li == 0 and h7_prev is not None:
                        # previous pair's SDF matmuls slot in here: they are
                        # ready to run (h7 relus done) and deepen both the
                        # L0->L1 and L7->SDF relu shadows by 4 matmuls.
                        psf_prev = emit_sdf(h7_prev)
                        h7_prev = None
                    if li == 1 and psf_prev is not None:
                        # drain early: the SDF banks are now in the shared
                        # ring and get reused ~8 allocations later
                        emit_sdf_out(p - 1, *psf_prev)
                        psf_prev = None
                    if li == 3:
                        h3 = h
                    if li == 5 and p + 1 < PAIRS:
                        # next pair's embedding: emitted here so its sins sit
                        # in the ACT queue ahead of the L6/L7 relus and
                        # complete well before the next pair's L0 matmuls.
                        emb_next = emit_embedding(p + 1)
                    h_prev = h

                h7_prev = h_prev
                embh = emb_next

            psfa, psfb = emit_sdf(h7_prev)
            emit_sdf_out(PAIRS - 1, psfa, psfb, split=True)
    nc.compile()
    return nc


def bass_ts(i, size):
    return slice(i * size, (i + 1) * size)


def _prep_maps(points, ws, bs, wsdf, bsdf):
    pts = np.ascontiguousarray(points, dtype=np.float32).reshape(N, 3)
    freqs = (2.0 ** np.arange(NHARM)).astype(np.float32)
    fcol18 = (np.repeat(freqs[None, :], 3, axis=0).reshape(18, 1) / TWO_PI).astype(
        np.float32
    )

    bmat = np.zeros((128, 16), dtype=np.float32)
    for i in range(8):
        for m in range(2):
            bmat[:, i * 2 + m] = bs[i][m * 128:(m + 1) * 128]

    wpack = np.zeros((128, WCOLS), dtype=np.float16)
    wpack[0:E, OFF_W0:OFF_W0 + 256] = ws[0].astype(np.float16)
    for li in _K_LAYERS:
        wpack[:, OFF_WK[li]:OFF_WK[li] + 256] = ws[li][0:128, :].astype(np.float16)
        wpack[:, OFF_WK[li] + 256:OFF_WK[li] + 512] = ws[li][128:256, :].astype(
            np.float16
        )
    wpack[0:E, OFF_W4E:OFF_W4E + 256] = ws[4][0:E, :].astype(np.float16)
    wpack[:, OFF_W4A:OFF_W4A + 256] = ws[4][E:E + 128, :].astype(np.float16)
    wpack[:, OFF_W4B:OFF_W4B + 256] = ws[4][E + 128:E + 256, :].astype(np.float16)
    wpack[:, OFF_SDF:OFF_SDF + 1] = wsdf[0:128, :].astype(np.float16)
    wpack[:, OFF_SDF + 1:OFF_SDF + 2] = wsdf[128:256, :].astype(np.float16)

    common = {
        "wpack": wpack,
        "bmat": bmat,
        "bsdf1": np.full((128, 1), float(np.ravel(bsdf)[0]), dtype=np.float32),
    }

    in_maps = []
    for c in range(N_CORES):
        sl = pts[c * NPC:(c + 1) * NPC]  # [NPC, 3]
        ptsT = np.ascontiguousarray(sl.T)  # [3, NPC]
        rep3 = np.repeat(ptsT, NHARM, axis=0)  # [18, NPC]
        t18 = rep3 * fcol18  # x * 2^j / (2pi), exact fp32 scaling
        t36 = np.empty((36, NPC), dtype=np.float32)
        t36[0:18], t36[18:36] = t18, t18 + np.float32(0.25)
        # host-side range reduction to [-0.5, 0.5] turns (same fp32 math the
        # kernel's DVE magic-round did); Sin LUT sees scale*y in [-pi, pi]
        rep6 = (t36 - np.round(t36)).astype(np.float16)
        m = dict(common)
        m["rep6"] = rep6
        m["ptsh"] = ptsT.astype(np.float16)
        in_maps.append(m)
    return in_maps


def kernel(
    points, w0, b0, w1, b1, w2, b2, w3, b3, w4, b4, w5, b5, w6, b6, w7, b7,
    wsdf, bsdf,
):
    ws = [np.asarray(w, dtype=np.float32) for w in (w0, w1, w2, w3, w4, w5, w6, w7)]
    bs = [np.asarray(b, dtype=np.float32) for b in (b0, b1, b2, b3, b4, b5, b6, b7)]
    in_maps = _prep_maps(
        np.asarray(points), ws, bs,
        np.asarray(wsdf, dtype=np.float32), np.asarray(bsdf, dtype=np.float32),
    )

    if "nc" not in _CACHED:
        _CACHED["nc"] = _build()
    nc = _CACHED["nc"]

    res = run_bass_kernel_spmd(nc, in_maps, core_ids=list(range(N_CORES)))
    out = np.concatenate(
        [res.results[c]["out_o"] for c in range(N_CORES)], axis=0
    ).reshape(N, 1).astype(np.float32)
    return out
